# Initial kernel scaffold
#
"""Bass/Trainium2 kernel for the BilinearInteractionLayer problem.

out[b, p, f] = (sum_e emb[b, I[p], e] * W[p, f, e]) * emb[b, J[p], f]
  emb: [2048, 40, 64] f32, W: [780, 64, 64] f32, out: [2048, 780, 64] f32

Strategy (data parallel over batch, 8 cores x 256 rows):
  - Pairs (i, j) grouped by i ("blocks"; block i has 39-i pairs, consecutive p).
    Blocks split into two 390-pair halves (A: i in 0..9 + 30..38, B: i in
    10..29) assigned to PE row-groups 0-63 / 64-127 so two K=64 matmuls run
    concurrently on the 128x128 array.
  - Per half, a "tape" of 390*64 = 24960 (pair, f) columns; W is pre-arranged
    on host to [128, 24960] (partition = e for half A rows 0-63 / half B rows
    64-127) and streamed in chunks of 2048 cols (~1 MB DMAs).
  - matmul: lhsT = embT[e, b] (stationary, [64, 128] per batch-chunk),
    rhs = W chunk slice [64, <=512], out psum[b, (pair, f)].
  - VectorE multiplies psum by emb[b, j, f] (contiguous slice of the natural
    layout) writing SBUF out tiles, DMA'd to HBM in tape order.
  - Host reorders tape pair order -> global pair order at the end.
"""

import os
import numpy as np

import concourse.mybir as mybir
import concourse.tile as tile
from concourse import bacc
from concourse import bass_utils

F32 = mybir.dt.float32

NUM_FIELDS = 40
EMBED = 64
BATCH = 2048
NCORES = 8
BL = BATCH // NCORES          # 256 rows per core
BCHUNKS = 2                   # 2 x 128 partition chunks of the local batch
NPAIRS = 780

HALVES = [list(range(0, 10)) + list(range(30, 39)), list(range(10, 30))]
HALF_PAIRS = 390
TAPE = HALF_PAIRS * EMBED     # 24960 cols per half
CHUNK = 2048                  # W/out tile width (cols); 12 full + 1 tail
NCHUNK = (TAPE + CHUNK - 1) // CHUNK
PSGRID = 1024                 # psum tile width (2 banks)
MMMAX = 512                   # max matmul free dim (one psum bank, fp32)


def _chunk_cols(c):
    return min(CHUNK, TAPE - c * CHUNK)


def _half_blocks(h):
    """[(i, tape_start_col, ncols)] for half h, in tape order."""
    res = []
    pos = 0
    for i in HALVES[h]:
        cols = (NUM_FIELDS - 1 - i) * EMBED
        res.append((i, pos, cols))
        pos += cols
    assert pos == TAPE
    return res


def _chunk_groups(h, c):
    """Groups for chunk c of half h: (i, abs_start, cols, j0).

    Split at block boundaries and at the PSGRID grid (relative to the chunk
    start) so each group fits one psum tile; j0 is the first j of the group.
    """
    c0, c1 = c * CHUNK, c * CHUNK + _chunk_cols(c)
    groups = []
    for (i, b0, bcols) in _half_blocks(h):
        lo, hi = max(b0, c0), min(b0 + bcols, c1)
        s = lo
        while s < hi:
            nxt = c0 + ((s - c0) // PSGRID + 1) * PSGRID
            e = min(hi, nxt)
            j0 = i + 1 + (s - b0) // EMBED
            groups.append((i, s, e - s, j0))
            s = e
    return groups


def _pairs_tape():
    """Global pair indices (combinations order) in tape order: half A then B."""
    pidx = {}
    k = 0
    for i in range(NUM_FIELDS):
        for j in range(i + 1, NUM_FIELDS):
            pidx[(i, j)] = k
            k += 1
    order = []
    for h in (0, 1):
        for i in HALVES[h]:
            for j in range(i + 1, NUM_FIELDS):
                order.append(pidx[(i, j)])
    return np.array(order, dtype=np.int64)


def _build_nc():
    nc = bacc.Bacc("TRN2", target_bir_lowering=False, debug=False)

    wt_d = nc.dram_tensor("Wt", [128, TAPE], F32, kind="ExternalInput")
    embT_d = nc.dram_tensor("embT", [64, NUM_FIELDS * BL], F32, kind="ExternalInput")
    embN_d = nc.dram_tensor("embN", [128, BCHUNKS * NUM_FIELDS * EMBED], F32,
                            kind="ExternalInput")
    out_d = nc.dram_tensor("out", [BL, 2 * TAPE], F32, kind="ExternalOutput")

    wt_ap, embT_ap, embN_ap, out_ap = (
        wt_d.ap(), embT_d.ap(), embN_d.ap(), out_d.ap())

    NF = NUM_FIELDS * EMBED  # 2560, embN cols per batch chunk

    with tile.TileContext(nc) as tc:
        with (
            tc.tile_pool(name="const", bufs=1) as cpool,
            tc.tile_pool(name="w", bufs=4) as wpool,
            tc.tile_pool(name="o", bufs=8) as opool,
            tc.tile_pool(name="ps", bufs=4, space="PSUM") as ppool,
        ):
            # Startup: load embT in two column halves (chunk-0 blocks only
            # need the first half) and duplicate each into partitions 64-127
            # on-chip (SBUF->SBUF, no HBM); embN rides the scalar ring in
            # parallel; W chunk 0 is emitted between the halves so the first
            # matmuls start as early as possible.
            hc = NUM_FIELDS * BL // 2
            embT_s = cpool.tile([128, NUM_FIELDS * BL], F32)
            nc.sync.dma_start(embT_s[0:64, :hc], embT_ap[:, :hc])
            nc.sync.dma_start(embT_s[64:128, :hc], embT_s[0:64, :hc])
            embN_s = cpool.tile([128, BCHUNKS * NF], F32)
            nc.scalar.dma_start(embN_s[:], embN_ap[:])
            first_w = wpool.tile([128, CHUNK], F32, tag="w", name="w_first")
            nc.sync.dma_start(first_w[:, :_chunk_cols(0)],
                              wt_ap[:, :_chunk_cols(0)])
            nc.sync.dma_start(embT_s[0:64, hc:], embT_ap[:, hc:])
            nc.sync.dma_start(embT_s[64:128, hc:], embT_s[0:64, hc:])

            for c in range(NCHUNK):
                ccols = _chunk_cols(c)
                if c == 0:
                    wt = first_w
                else:
                    wt = wpool.tile([128, CHUNK], F32, tag="w")
                    nc.sync.dma_start(wt[:, :ccols],
                                      wt_ap[:, c * CHUNK:c * CHUNK + ccols])
                groups_h = [_chunk_groups(0, c), _chunk_groups(1, c)]
                for bc in range(BCHUNKS):
                    otiles = [opool.tile([128, CHUNK], F32, tag="o", name=f"o{c}_{bc}_{h}")
                              for h in range(2)]

                    def emit_half(h):
                        for (i, gs, gcols, j0) in groups_h[h]:
                            pt = ppool.tile([128, PSGRID], F32, tag="ps",
                                            name=f"ps{c}_{bc}_{h}_{gs}")
                            s = 0
                            while s < gcols:
                                w = min(MMMAX, gcols - s)
                                yield ("mm", (h, pt, i, gs, s, w))
                                s += w
                            yield ("mul", (h, pt, gs, gcols, j0))

                    streams = [emit_half(0), emit_half(1)]
                    done = [False, False]
                    turn = 0
                    while not all(done):
                        if done[turn]:
                            turn ^= 1
                        try:
                            kind, args = next(streams[turn])
                        except StopIteration:
                            done[turn] = True
                            turn ^= 1
                            continue
                        if kind == "mm":
                            h, pt, i, gs, s, w = args
                            rel = gs - c * CHUNK
                            col0 = i * BL + bc * 128
                            nc.tensor.matmul(
                                pt[:, s:s + w],
                                lhsT=embT_s[h * 64:(h + 1) * 64, col0:col0 + 128],
                                rhs=wt[h * 64:(h + 1) * 64, rel + s:rel + s + w],
                                start=True, stop=True,
                            )
                            # alternate halves between matmuls for row-group
                            # concurrency on the PE array
                            turn ^= 1
                        else:
                            h, pt, gs, gcols, j0 = args
                            rel = gs - c * CHUNK
                            nc.vector.tensor_mul(
                                otiles[h][:, rel:rel + gcols],
                                pt[:, :gcols],
                                embN_s[:, bc * NF + j0 * EMBED:
                                       bc * NF + j0 * EMBED + gcols],
                            )
                    for h in range(2):
                        nc.scalar.dma_start(
                            out_ap[bc * 128:(bc + 1) * 128,
                                   h * TAPE + c * CHUNK:
                                   h * TAPE + c * CHUNK + ccols],
                            otiles[h][:, :ccols],
                        )

    nc.compile()
    return nc


_NC = None
_TAPE_ORDER = None
LAST_RESULT = None


def kernel(feature_emb, W):
    global _NC, _TAPE_ORDER, LAST_RESULT
    feature_emb = np.ascontiguousarray(feature_emb, dtype=np.float32)
    W = np.ascontiguousarray(W, dtype=np.float32)
    assert feature_emb.shape == (BATCH, NUM_FIELDS, EMBED)
    assert W.shape == (NPAIRS, EMBED, EMBED)

    if _NC is None:
        _NC = _build_nc()
        _TAPE_ORDER = _pairs_tape()

    # W tape: [128, 24960]; rows 0-63 half A (partition = e), rows 64-127 half B
    wsel = W[_TAPE_ORDER]                       # [780, 64(f), 64(e)] tape order
    wa = wsel[:HALF_PAIRS].transpose(2, 0, 1).reshape(EMBED, TAPE)
    wb = wsel[HALF_PAIRS:].transpose(2, 0, 1).reshape(EMBED, TAPE)
    wt = np.ascontiguousarray(np.concatenate([wa, wb], axis=0))

    in_maps = []
    for c in range(NCORES):
        ec = feature_emb[c * BL:(c + 1) * BL]   # [256, 40, 64]
        embT = np.ascontiguousarray(
            ec.transpose(2, 1, 0).reshape(EMBED, NUM_FIELDS * BL))  # [e,(i,b)]
        embN = np.ascontiguousarray(
            ec.reshape(BCHUNKS, 128, NUM_FIELDS * EMBED)
              .transpose(1, 0, 2).reshape(128, BCHUNKS * NUM_FIELDS * EMBED))
        in_maps.append({"Wt": wt, "embT": embT, "embN": embN})

    trace = bool(int(os.environ.get("BILIN_TRACE", "0")))
    res = bass_utils.run_bass_kernel_spmd(
        _NC, in_maps, core_ids=list(range(NCORES)), trace=trace)
    LAST_RESULT = res

    out = np.empty((BATCH, NPAIRS, EMBED), dtype=np.float32)
    for c in range(NCORES):
        t = np.asarray(res.results[c]["out"]).reshape(BL, NPAIRS, EMBED)
        out[c * BL:(c + 1) * BL][:, _TAPE_ORDER, :] = t
    return out



# revision 42
# speedup vs baseline: 1.5730x; 1.5730x over previous
"""Bass/Trainium2 kernel for the BilinearInteractionLayer problem.

out[b, p, f] = (sum_e emb[b, I[p], e] * W[p, f, e]) * emb[b, J[p], f]
  emb: [2048, 40, 64] f32, W: [780, 64, 64] f32, out: [2048, 780, 64] f32

Strategy (data parallel over batch, 8 cores x 256 rows), bf16 compute:
  - All operands cast to bf16 on host; matmul accumulates fp32 in PSUM; the
    output is written bf16 and upcast to fp32 on host.  This halves every
    HBM stream (out 25.6 MB, W 6.4 MB, emb 2.6 MB per core) and the kernel
    is HBM-bound, so bytes ~= time.
  - Pairs (i, j) grouped by i ("blocks"; block i has 39-i pairs, consecutive p).
    Blocks split into two 390-pair halves (A: i in 0..9 + 30..38, B: i in
    10..29) assigned to PE row-groups 0-63 / 64-127 so two K=64 matmuls can
    overlap on the 128x128 array.
  - Per half, a "tape" of 390*64 = 24960 (pair, f) columns; W is pre-arranged
    on host to [128, 24960] bf16 (partition = e; half A rows 0-63, half B rows
    64-127) and streamed in 6 chunks of 4160 cols (~1 MB DMAs).
  - matmul: lhsT = embT[e, b] (stationary, [64, 128] per batch-chunk),
    rhs = W chunk slice [64, <=512], out psum[b, (pair, f)] fp32; halves
    alternate so one half's LDWEIGHTS overlaps the other's pipe drain.
  - Epilogue split across three engines so no engine exceeds the DMA floor,
    assigned per block-run by a deficit balance: a run either gets DVE
    tensor_muls straight out of PSUM (fp32 x bf16 embN -> bf16 out tile), or
    ACT casts (PSUM -> bf16 out tile) with one merged in-place GpSimd bf16
    multiply; the last run of each out tile goes to DVE so the store isn't
    gated by a late GpSimd flush.
  - Out tiles [128, 4160] bf16 DMA to HBM in tape order; host reorders
    tape pair order -> global pair order and upcasts.
"""

import os
import numpy as np
import ml_dtypes

import concourse.mybir as mybir
import concourse.tile as tile
from concourse import bacc
from concourse import bass_utils

F32 = mybir.dt.float32
BF16 = mybir.dt.bfloat16
NPBF16 = ml_dtypes.bfloat16

NUM_FIELDS = 40
EMBED = 64
BATCH = 2048
NCORES = 8
BL = BATCH // NCORES          # 256 rows per core
BCHUNKS = 2                   # 2 x 128 partition chunks of the local batch
NPAIRS = 780

HALVES = [list(range(0, 10)) + list(range(30, 39)), list(range(10, 30))]
HALF_PAIRS = 390
TAPE = HALF_PAIRS * EMBED     # 24960 cols per half
CHUNK = 4160                  # W/out tile width (cols); 6 even chunks
NCHUNK = TAPE // CHUNK
PSGRID = 1024                 # psum tile width (2 banks, fp32)
MMMAX = 512                   # max matmul free dim (one psum bank, fp32 out)

# Epilogue path split (fractions of output columns), tuned from measured
# engine rates: DVE direct-psum multiply ~1.3 ns/col, ACT cast ~1.1,
# GpSimd bf16 multiply ~2.05 (the GPS path also costs an ACT cast).
TARGET = {"dve": 0.625, "gps": 0.375}


def _half_blocks(h):
    """[(i, tape_start_col, ncols)] for half h, in tape order."""
    res = []
    pos = 0
    for i in HALVES[h]:
        cols = (NUM_FIELDS - 1 - i) * EMBED
        res.append((i, pos, cols))
        pos += cols
    assert pos == TAPE
    return res


def _chunk_groups(h, c):
    """Groups for chunk c of half h: (i, abs_start, cols, j0).

    Split at block boundaries and at the PSGRID grid (relative to the chunk
    start) so each group fits one psum tile; j0 is the first j of the group.
    All boundaries are multiples of 64.
    """
    c0, c1 = c * CHUNK, (c + 1) * CHUNK
    groups = []
    for (i, b0, bcols) in _half_blocks(h):
        lo, hi = max(b0, c0), min(b0 + bcols, c1)
        s = lo
        while s < hi:
            nxt = c0 + ((s - c0) // PSGRID + 1) * PSGRID
            e = min(hi, nxt)
            j0 = i + 1 + (s - b0) // EMBED
            groups.append((i, s, e - s, j0))
            s = e
    return groups


def _chunk_runs(h, c):
    """Pieces of chunk c of half h grouped into per-block runs.

    Returns [ [ (i, abs_start, cols, j0), ... ], ... ] where each inner list
    is the PSGRID-split pieces of one block(cap chunk) in tape order (so the
    run's columns are contiguous in both the out tile and embN).
    """
    runs = []
    for g in _chunk_groups(h, c):
        if runs and runs[-1][0][0] == g[0]:
            runs[-1].append(g)
        else:
            runs.append([g])
    return runs


def _pairs_tape():
    """Global pair indices (combinations order) in tape order: half A then B."""
    pidx = {}
    k = 0
    for i in range(NUM_FIELDS):
        for j in range(i + 1, NUM_FIELDS):
            pidx[(i, j)] = k
            k += 1
    order = []
    for h in (0, 1):
        for i in HALVES[h]:
            for j in range(i + 1, NUM_FIELDS):
                order.append(pidx[(i, j)])
    return np.array(order, dtype=np.int64)


def _build_nc():
    nc = bacc.Bacc("TRN2", target_bir_lowering=False, debug=False)

    wt_d = nc.dram_tensor("Wt", [128, TAPE], BF16, kind="ExternalInput")
    # embT is staged host-side duplicated into both partition halves so a
    # single full-128-partition DMA loads it (the compiler requires lhsT and
    # rhs on the same partitions, and 64-partition DMAs run at half rate).
    embT_d = nc.dram_tensor("embT", [128, NUM_FIELDS * BL], BF16,
                            kind="ExternalInput")
    embN_d = nc.dram_tensor("embN", [128, BCHUNKS * NUM_FIELDS * EMBED], BF16,
                            kind="ExternalInput")
    out_d = nc.dram_tensor("out", [BL, 2 * TAPE], BF16, kind="ExternalOutput")

    wt_ap, embT_ap, embN_ap, out_ap = (
        wt_d.ap(), embT_d.ap(), embN_d.ap(), out_d.ap())

    NF = NUM_FIELDS * EMBED  # 2560, embN cols per batch chunk

    with tile.TileContext(nc) as tc:
        with (
            tc.tile_pool(name="const", bufs=1) as cpool,
            tc.tile_pool(name="w", bufs=6) as wpool,
            tc.tile_pool(name="o", bufs=8) as opool,
            tc.tile_pool(name="ps", bufs=4, space="PSUM") as ppool,
        ):
            # Startup: load W chunk 0 and embT in small "starter" segments so
            # the first matmuls fire as soon as ~1 MB has landed.  Everything
            # not needed in the first two chunks (W2-W5, the right embT half)
            # is prefetched at t=0 on the scalar ring — loads have no input
            # deps, so they can't head-of-line block the casts that follow on
            # that queue — leaving the sync ring to carry almost only the out
            # stores; the two rings hide each other's per-transfer gaps.
            hc = NUM_FIELDS * BL // 2
            w0a = wpool.tile([128, 2048], BF16, tag="w", name="w0a")
            nc.sync.dma_start(w0a[:], wt_ap[:, :2048])
            eTa = cpool.tile([128, 1024], BF16, name="eTa")
            nc.sync.dma_start(eTa[:], embT_ap[:, :1024])
            embN_s = cpool.tile([128, BCHUNKS * NF], BF16)
            nc.scalar.dma_start(embN_s[:], embN_ap[:])
            w0b = wpool.tile([128, CHUNK - 2048], BF16, tag="w", name="w0b")
            nc.sync.dma_start(w0b[:], wt_ap[:, 2048:CHUNK])
            eTb = cpool.tile([128, hc - 1024], BF16, name="eTb")
            nc.sync.dma_start(eTb[:], embT_ap[:, 1024:hc])
            wts = {0: [(w0a, 0, 2048), (w0b, 2048, CHUNK)]}
            wts[1] = [(wpool.tile([128, CHUNK], BF16, tag="w", name="wt"),
                       0, CHUNK)]
            nc.sync.dma_start(wts[1][0][0][:], wt_ap[:, CHUNK:2 * CHUNK])
            eTc = cpool.tile([128, hc], BF16, name="eTc")
            for c in range(2, NCHUNK):
                wts[c] = [(wpool.tile([128, CHUNK], BF16, tag="w", name="wt"),
                           0, CHUNK)]
                nc.scalar.dma_start(wts[c][0][0][:],
                                    wt_ap[:, c * CHUNK:(c + 1) * CHUNK])
            nc.scalar.dma_start(eTc[:], embT_ap[:, hc:])
            embT_segs = [(eTa, 0, 1024), (eTb, 1024, hc), (eTc, hc, 2 * hc)]

            def lhsT_slice(h, col0):
                for (t, lo, hi) in embT_segs:
                    if lo <= col0 < hi:
                        return t[h * 64:(h + 1) * 64, col0 - lo:col0 - lo + 128]
                raise AssertionError(col0)

            def wt_slice(segs, h, lo, width):
                for (t, slo, shi) in segs:
                    if slo <= lo < shi:
                        return t[h * 64:(h + 1) * 64, lo - slo:lo - slo + width]
                raise AssertionError(lo)

            # Deficit state for the epilogue path balance.
            got = {k: 0.0 for k in TARGET}
            tot = [0.0]

            for c in range(NCHUNK):
                wt_segs = wts.pop(c)
                runs_h = [_chunk_runs(0, c), _chunk_runs(1, c)]
                for bc in range(BCHUNKS):
                    otiles = [opool.tile([128, CHUNK], BF16, tag="o",
                                         name=f"o{c}_{bc}_{h}")
                              for h in range(2)]

                    def emit_half(h):
                        # Global deficit balance toward TARGET fractions so
                        # DVE and the ACT+GpSimd chain run concurrently; the
                        # last run of each out tile is forced to DVE so the
                        # tile's store isn't gated by a late GpSimd flush.
                        nruns = len(runs_h[h])
                        for ridx, run in enumerate(runs_h[h]):
                            rcols = sum(g[2] for g in run)
                            tot[0] += rcols
                            if ridx == nruns - 1:
                                path = "dve"
                            else:
                                path = max(TARGET, key=lambda k:
                                           TARGET[k] * tot[0] - got[k])
                            got[path] += rcols
                            for (i, gs, gcols, j0) in run:
                                pt = ppool.tile([128, PSGRID], F32, tag="ps",
                                                name=f"ps{c}_{bc}_{h}_{gs}")
                                s = 0
                                while s < gcols:
                                    w = min(MMMAX, gcols - s)
                                    yield ("mm", (h, pt, i, gs, s, w))
                                    s += w
                                yield ("epi", (h, pt, gs, gcols, j0, path))
                            if path != "dve":
                                yield ("flush",
                                       (h, run[0][1], rcols, run[0][3]))

                    streams = [emit_half(0), emit_half(1)]
                    done = [False, False]
                    turn = 0
                    while not all(done):
                        if done[turn]:
                            turn ^= 1
                        try:
                            kind, args = next(streams[turn])
                        except StopIteration:
                            done[turn] = True
                            turn ^= 1
                            continue
                        if kind == "mm":
                            h, pt, i, gs, s, w = args
                            rel = gs - c * CHUNK
                            col0 = i * BL + bc * 128
                            nc.tensor.matmul(
                                pt[:, s:s + w],
                                lhsT=lhsT_slice(h, col0),
                                rhs=wt_slice(wt_segs, h, rel + s, w),
                                start=True, stop=True,
                            )
                            # alternate halves between matmuls so LDWEIGHTS
                            # for one PE row-group overlaps the other's drain
                            turn ^= 1
                        elif kind == "epi":
                            h, pt, gs, gcols, j0, path = args
                            rel = gs - c * CHUNK
                            ncol = bc * NF + j0 * EMBED
                            ot = otiles[h]
                            if path == "dve":
                                nc.vector.tensor_mul(
                                    ot[:, rel:rel + gcols],
                                    pt[:, :gcols],
                                    embN_s[:, ncol:ncol + gcols],
                                )
                            else:
                                nc.scalar.copy(
                                    ot[:, rel:rel + gcols],
                                    pt[:, :gcols],
                                )
                        else:
                            h, gs0, rcols, j0 = args
                            rel = gs0 - c * CHUNK
                            ncol = bc * NF + j0 * EMBED
                            ot = otiles[h]
                            nc.gpsimd.tensor_mul(
                                ot[:, rel:rel + rcols],
                                ot[:, rel:rel + rcols],
                                embN_s[:, ncol:ncol + rcols],
                            )
                    last_win = c == NCHUNK - 1 and bc == BCHUNKS - 1
                    for h in range(2):
                        # out stores live on the sync ring (an issue on the
                        # ACT ring would head-of-line block later casts);
                        # the final window's tiles split across both rings
                        # in halves so the tail drains in parallel.
                        dst = out_ap[bc * 128:(bc + 1) * 128,
                                     h * TAPE + c * CHUNK:
                                     h * TAPE + (c + 1) * CHUNK]
                        if last_win:
                            hw = CHUNK // 2
                            nc.sync.dma_start(dst[:, :hw], otiles[h][:, :hw])
                            nc.scalar.dma_start(dst[:, hw:], otiles[h][:, hw:])
                        else:
                            nc.sync.dma_start(dst, otiles[h][:])

    nc.compile()
    return nc


_NC = None
_TAPE_ORDER = None
LAST_RESULT = None


def kernel(feature_emb, W):
    global _NC, _TAPE_ORDER, LAST_RESULT
    feature_emb = np.ascontiguousarray(feature_emb, dtype=np.float32)
    W = np.ascontiguousarray(W, dtype=np.float32)
    assert feature_emb.shape == (BATCH, NUM_FIELDS, EMBED)
    assert W.shape == (NPAIRS, EMBED, EMBED)

    if _NC is None:
        _NC = _build_nc()
        _TAPE_ORDER = _pairs_tape()

    # W tape: [128, 24960] bf16; rows 0-63 half A (partition = e), 64-127 half B
    wsel = W[_TAPE_ORDER]                       # [780, 64(f), 64(e)] tape order
    wa = wsel[:HALF_PAIRS].transpose(2, 0, 1).reshape(EMBED, TAPE)
    wb = wsel[HALF_PAIRS:].transpose(2, 0, 1).reshape(EMBED, TAPE)
    wt = np.ascontiguousarray(
        np.concatenate([wa, wb], axis=0)).astype(NPBF16)

    in_maps = []
    for c in range(NCORES):
        ec = feature_emb[c * BL:(c + 1) * BL]   # [256, 40, 64]
        embT1 = ec.transpose(2, 1, 0).reshape(EMBED, NUM_FIELDS * BL)
        embT = np.ascontiguousarray(
            np.concatenate([embT1, embT1], axis=0)).astype(NPBF16)
        embN = np.ascontiguousarray(
            ec.reshape(BCHUNKS, 128, NUM_FIELDS * EMBED)
              .transpose(1, 0, 2)
              .reshape(128, BCHUNKS * NUM_FIELDS * EMBED)).astype(NPBF16)
        in_maps.append({"Wt": wt, "embT": embT, "embN": embN})

    trace = bool(int(os.environ.get("BILIN_TRACE", "0")))
    res = bass_utils.run_bass_kernel_spmd(
        _NC, in_maps, core_ids=list(range(NCORES)), trace=trace)
    LAST_RESULT = res

    out = np.empty((BATCH, NPAIRS, EMBED), dtype=np.float32)
    for c in range(NCORES):
        t = np.asarray(res.results[c]["out"]).astype(np.float32)
        t = t.reshape(BL, NPAIRS, EMBED)
        out[c * BL:(c + 1) * BL][:, _TAPE_ORDER, :] = t
    return out


# revision 44
# speedup vs baseline: 1.6499x; 1.0489x over previous
"""Bass/Trainium2 kernel for the BilinearInteractionLayer problem.

out[b, p, f] = (sum_e emb[b, I[p], e] * W[p, f, e]) * emb[b, J[p], f]
  emb: [2048, 40, 64] f32, W: [780, 64, 64] f32, out: [2048, 780, 64] f32

Strategy (data parallel over batch, 8 cores x 256 rows), bf16 compute:
  - All operands cast to bf16 on host; matmul accumulates fp32 in PSUM; the
    output is written bf16 and upcast to fp32 on host.  This halves every
    HBM stream (out 25.6 MB, W 6.4 MB, emb 2.6 MB per core) and the kernel
    is HBM-bound, so bytes ~= time.
  - Pairs (i, j) grouped by i ("blocks"; block i has 39-i pairs, consecutive p).
    Blocks split into two 390-pair halves (A: i in 0..9 + 30..38, B: i in
    10..29) assigned to PE row-groups 0-63 / 64-127 so two K=64 matmuls can
    overlap on the 128x128 array.
  - Per half, a "tape" of 390*64 = 24960 (pair, f) columns; W is pre-arranged
    on host to [128, 24960] bf16 (partition = e; half A rows 0-63, half B rows
    64-127) and streamed in 6 chunks of 4160 cols (~1 MB DMAs).
  - matmul: lhsT = embT[e, b] (stationary, [64, 128] per batch-chunk),
    rhs = W chunk slice [64, <=512], out psum[b, (pair, f)] fp32; halves
    alternate so one half's LDWEIGHTS overlaps the other's pipe drain.
  - Epilogue split across three engines so no engine exceeds the DMA floor,
    assigned per block-run by a deficit balance: a run either gets DVE
    tensor_muls straight out of PSUM (fp32 x bf16 embN -> bf16 out tile), or
    ACT casts (PSUM -> bf16 out tile) with one merged in-place GpSimd bf16
    multiply; the last run of each out tile goes to DVE so the store isn't
    gated by a late GpSimd flush.
  - Out tiles [128, 4160] bf16 DMA to HBM in tape order; host reorders
    tape pair order -> global pair order and upcasts.
"""

import os
import numpy as np
import ml_dtypes

import concourse.mybir as mybir
import concourse.tile as tile
from concourse import bacc
from concourse import bass_utils

F32 = mybir.dt.float32
BF16 = mybir.dt.bfloat16
NPBF16 = ml_dtypes.bfloat16

NUM_FIELDS = 40
EMBED = 64
BATCH = 2048
NCORES = 8
BL = BATCH // NCORES          # 256 rows per core
BCHUNKS = 2                   # 2 x 128 partition chunks of the local batch
NPAIRS = 780

HALVES = [list(range(0, 10)) + list(range(30, 39)), list(range(10, 30))]
HALF_PAIRS = 390
TAPE = HALF_PAIRS * EMBED     # 24960 cols per half
CHUNK = 4160                  # W/out tile width (cols); 6 even chunks
NCHUNK = TAPE // CHUNK
PSGRID = 1024                 # psum tile width (2 banks, fp32)
MMMAX = 512                   # max matmul free dim (one psum bank, fp32 out)

# Epilogue path split (fractions of output columns), tuned from measured
# engine rates: DVE direct-psum multiply ~1.3 ns/col, ACT cast ~1.1,
# GpSimd bf16 multiply ~2.05 (the GPS path also costs an ACT cast).
TARGET = {"dve": 0.625, "gps": 0.375}


def _half_blocks(h):
    """[(i, tape_start_col, ncols)] for half h, in tape order."""
    res = []
    pos = 0
    for i in HALVES[h]:
        cols = (NUM_FIELDS - 1 - i) * EMBED
        res.append((i, pos, cols))
        pos += cols
    assert pos == TAPE
    return res


def _chunk_groups(h, c):
    """Groups for chunk c of half h: (i, abs_start, cols, j0).

    Split at block boundaries and at the PSGRID grid (relative to the chunk
    start) so each group fits one psum tile; j0 is the first j of the group.
    All boundaries are multiples of 64.
    """
    c0, c1 = c * CHUNK, (c + 1) * CHUNK
    groups = []
    for (i, b0, bcols) in _half_blocks(h):
        lo, hi = max(b0, c0), min(b0 + bcols, c1)
        s = lo
        while s < hi:
            nxt = c0 + ((s - c0) // PSGRID + 1) * PSGRID
            e = min(hi, nxt)
            j0 = i + 1 + (s - b0) // EMBED
            groups.append((i, s, e - s, j0))
            s = e
    return groups


def _chunk_runs(h, c):
    """Pieces of chunk c of half h grouped into per-block runs.

    Returns [ [ (i, abs_start, cols, j0), ... ], ... ] where each inner list
    is the PSGRID-split pieces of one block(cap chunk) in tape order (so the
    run's columns are contiguous in both the out tile and embN).
    """
    runs = []
    for g in _chunk_groups(h, c):
        if runs and runs[-1][0][0] == g[0]:
            runs[-1].append(g)
        else:
            runs.append([g])
    return runs


def _pairs_tape():
    """Global pair indices (combinations order) in tape order: half A then B."""
    pidx = {}
    k = 0
    for i in range(NUM_FIELDS):
        for j in range(i + 1, NUM_FIELDS):
            pidx[(i, j)] = k
            k += 1
    order = []
    for h in (0, 1):
        for i in HALVES[h]:
            for j in range(i + 1, NUM_FIELDS):
                order.append(pidx[(i, j)])
    return np.array(order, dtype=np.int64)


def _build_nc():
    nc = bacc.Bacc("TRN2", target_bir_lowering=False, debug=False)

    wt_d = nc.dram_tensor("Wt", [128, TAPE], BF16, kind="ExternalInput")
    # embT is staged host-side duplicated into both partition halves so a
    # single full-128-partition DMA loads it (the compiler requires lhsT and
    # rhs on the same partitions, and 64-partition DMAs run at half rate).
    embT_d = nc.dram_tensor("embT", [128, NUM_FIELDS * BL], BF16,
                            kind="ExternalInput")
    embN_d = nc.dram_tensor("embN", [128, BCHUNKS * NUM_FIELDS * EMBED], BF16,
                            kind="ExternalInput")
    out_d = nc.dram_tensor("out", [BL, 2 * TAPE], BF16, kind="ExternalOutput")

    wt_ap, embT_ap, embN_ap, out_ap = (
        wt_d.ap(), embT_d.ap(), embN_d.ap(), out_d.ap())

    NF = NUM_FIELDS * EMBED  # 2560, embN cols per batch chunk

    with tile.TileContext(nc) as tc:
        with (
            tc.tile_pool(name="const", bufs=1) as cpool,
            tc.tile_pool(name="w", bufs=4) as wpool,
            tc.tile_pool(name="o", bufs=8) as opool,
            tc.tile_pool(name="ps", bufs=4, space="PSUM") as ppool,
        ):
            # Startup: load W chunk 0 and embT in small "starter" segments so
            # the first matmuls fire as soon as ~1 MB has landed, instead of
            # waiting for the full 2.3 MB.  Ring order interleaves W and embT
            # segments in first-use order; embN rides the scalar ring in
            # parallel.  W chunk c+2 is issued ahead of chunk c's out stores
            # for a one-chunk prefetch runway (the interleaved loads also
            # fill store-dependency gaps on the ring).
            hc = NUM_FIELDS * BL // 2
            w0a = wpool.tile([128, 2048], BF16, tag="w", name="w0a")
            nc.sync.dma_start(w0a[:], wt_ap[:, :2048])
            eTa = cpool.tile([128, 1024], BF16, name="eTa")
            nc.sync.dma_start(eTa[:], embT_ap[:, :1024])
            embN_s = cpool.tile([128, BCHUNKS * NF], BF16)
            nc.scalar.dma_start(embN_s[:], embN_ap[:])
            w0b = wpool.tile([128, CHUNK - 2048], BF16, tag="w", name="w0b")
            nc.sync.dma_start(w0b[:], wt_ap[:, 2048:CHUNK])
            eTb = cpool.tile([128, hc - 1024], BF16, name="eTb")
            nc.sync.dma_start(eTb[:], embT_ap[:, 1024:hc])
            wts = {0: [(w0a, 0, 2048), (w0b, 2048, CHUNK)]}
            wts[1] = [(wpool.tile([128, CHUNK], BF16, tag="w", name="wt"),
                       0, CHUNK)]
            nc.sync.dma_start(wts[1][0][0][:], wt_ap[:, CHUNK:2 * CHUNK])
            eTc = cpool.tile([128, hc], BF16, name="eTc")
            nc.sync.dma_start(eTc[:], embT_ap[:, hc:])
            embT_segs = [(eTa, 0, 1024), (eTb, 1024, hc), (eTc, hc, 2 * hc)]

            def lhsT_slice(h, col0):
                for (t, lo, hi) in embT_segs:
                    if lo <= col0 < hi:
                        return t[h * 64:(h + 1) * 64, col0 - lo:col0 - lo + 128]
                raise AssertionError(col0)

            def wt_slice(segs, h, lo, width):
                for (t, slo, shi) in segs:
                    if slo <= lo < shi:
                        return t[h * 64:(h + 1) * 64, lo - slo:lo - slo + width]
                raise AssertionError(lo)

            # Deficit state for the epilogue path balance.
            got = {k: 0.0 for k in TARGET}
            tot = [0.0]

            for c in range(NCHUNK):
                wt_segs = wts.pop(c)
                if c + 2 < NCHUNK:
                    wnew = wpool.tile([128, CHUNK], BF16, tag="w", name="wt")
                    wts[c + 2] = [(wnew, 0, CHUNK)]
                    nc.sync.dma_start(wnew[:],
                                      wt_ap[:, (c + 2) * CHUNK:(c + 3) * CHUNK])
                if c == NCHUNK - 1:
                    # final chunk: break run-merging so pieces spread across
                    # engines at fine grain and the last tiles finish sooner
                    runs_h = [[[g] for g in _chunk_groups(0, c)],
                              [[g] for g in _chunk_groups(1, c)]]
                else:
                    runs_h = [_chunk_runs(0, c), _chunk_runs(1, c)]
                for bc in range(BCHUNKS):
                    otiles = [opool.tile([128, CHUNK], BF16, tag="o",
                                         name=f"o{c}_{bc}_{h}")
                              for h in range(2)]

                    def emit_half(h):
                        # Global deficit balance toward TARGET fractions so
                        # DVE and the ACT+GpSimd chain run concurrently; the
                        # last run of each out tile is forced to DVE so the
                        # tile's store isn't gated by a late GpSimd flush.
                        nruns = len(runs_h[h])
                        for ridx, run in enumerate(runs_h[h]):
                            rcols = sum(g[2] for g in run)
                            tot[0] += rcols
                            if ridx == nruns - 1:
                                path = "dve"
                            else:
                                path = max(TARGET, key=lambda k:
                                           TARGET[k] * tot[0] - got[k])
                            got[path] += rcols
                            for (i, gs, gcols, j0) in run:
                                pt = ppool.tile([128, PSGRID], F32, tag="ps",
                                                name=f"ps{c}_{bc}_{h}_{gs}")
                                s = 0
                                while s < gcols:
                                    w = min(MMMAX, gcols - s)
                                    yield ("mm", (h, pt, i, gs, s, w))
                                    s += w
                                yield ("epi", (h, pt, gs, gcols, j0, path))
                            if path != "dve":
                                yield ("flush",
                                       (h, run[0][1], rcols, run[0][3]))

                    streams = [emit_half(0), emit_half(1)]
                    done = [False, False]
                    turn = 0
                    while not all(done):
                        if done[turn]:
                            turn ^= 1
                        try:
                            kind, args = next(streams[turn])
                        except StopIteration:
                            done[turn] = True
                            turn ^= 1
                            continue
                        if kind == "mm":
                            h, pt, i, gs, s, w = args
                            rel = gs - c * CHUNK
                            col0 = i * BL + bc * 128
                            nc.tensor.matmul(
                                pt[:, s:s + w],
                                lhsT=lhsT_slice(h, col0),
                                rhs=wt_slice(wt_segs, h, rel + s, w),
                                start=True, stop=True,
                            )
                            # alternate halves between matmuls so LDWEIGHTS
                            # for one PE row-group overlaps the other's drain
                            turn ^= 1
                        elif kind == "epi":
                            h, pt, gs, gcols, j0, path = args
                            rel = gs - c * CHUNK
                            ncol = bc * NF + j0 * EMBED
                            ot = otiles[h]
                            if path == "dve":
                                nc.vector.tensor_mul(
                                    ot[:, rel:rel + gcols],
                                    pt[:, :gcols],
                                    embN_s[:, ncol:ncol + gcols],
                                )
                            else:
                                nc.scalar.copy(
                                    ot[:, rel:rel + gcols],
                                    pt[:, :gcols],
                                )
                        else:
                            h, gs0, rcols, j0 = args
                            rel = gs0 - c * CHUNK
                            ncol = bc * NF + j0 * EMBED
                            ot = otiles[h]
                            nc.gpsimd.tensor_mul(
                                ot[:, rel:rel + rcols],
                                ot[:, rel:rel + rcols],
                                embN_s[:, ncol:ncol + rcols],
                            )
                    last_win = c == NCHUNK - 1 and bc == BCHUNKS - 1
                    for h in range(2):
                        # out stores live on the sync ring (an issue on the
                        # ACT ring would head-of-line block later casts);
                        # the final window's tiles split across both rings
                        # in halves so the tail drains in parallel.
                        dst = out_ap[bc * 128:(bc + 1) * 128,
                                     h * TAPE + c * CHUNK:
                                     h * TAPE + (c + 1) * CHUNK]
                        if last_win:
                            hw = CHUNK // 2
                            nc.sync.dma_start(dst[:, :hw], otiles[h][:, :hw])
                            nc.scalar.dma_start(dst[:, hw:], otiles[h][:, hw:])
                        else:
                            nc.sync.dma_start(dst, otiles[h][:])

    nc.compile()
    return nc


_NC = None
_TAPE_ORDER = None
LAST_RESULT = None


def kernel(feature_emb, W):
    global _NC, _TAPE_ORDER, LAST_RESULT
    feature_emb = np.ascontiguousarray(feature_emb, dtype=np.float32)
    W = np.ascontiguousarray(W, dtype=np.float32)
    assert feature_emb.shape == (BATCH, NUM_FIELDS, EMBED)
    assert W.shape == (NPAIRS, EMBED, EMBED)

    if _NC is None:
        _NC = _build_nc()
        _TAPE_ORDER = _pairs_tape()

    # W tape: [128, 24960] bf16; rows 0-63 half A (partition = e), 64-127 half B
    wsel = W[_TAPE_ORDER]                       # [780, 64(f), 64(e)] tape order
    wa = wsel[:HALF_PAIRS].transpose(2, 0, 1).reshape(EMBED, TAPE)
    wb = wsel[HALF_PAIRS:].transpose(2, 0, 1).reshape(EMBED, TAPE)
    wt = np.ascontiguousarray(
        np.concatenate([wa, wb], axis=0)).astype(NPBF16)

    in_maps = []
    for c in range(NCORES):
        ec = feature_emb[c * BL:(c + 1) * BL]   # [256, 40, 64]
        embT1 = ec.transpose(2, 1, 0).reshape(EMBED, NUM_FIELDS * BL)
        embT = np.ascontiguousarray(
            np.concatenate([embT1, embT1], axis=0)).astype(NPBF16)
        embN = np.ascontiguousarray(
            ec.reshape(BCHUNKS, 128, NUM_FIELDS * EMBED)
              .transpose(1, 0, 2)
              .reshape(128, BCHUNKS * NUM_FIELDS * EMBED)).astype(NPBF16)
        in_maps.append({"Wt": wt, "embT": embT, "embN": embN})

    trace = bool(int(os.environ.get("BILIN_TRACE", "0")))
    res = bass_utils.run_bass_kernel_spmd(
        _NC, in_maps, core_ids=list(range(NCORES)), trace=trace)
    LAST_RESULT = res

    out = np.empty((BATCH, NPAIRS, EMBED), dtype=np.float32)
    for c in range(NCORES):
        t = np.asarray(res.results[c]["out"]).astype(np.float32)
        t = t.reshape(BL, NPAIRS, EMBED)
        out[c * BL:(c + 1) * BL][:, _TAPE_ORDER, :] = t
    return out


# revision 45
# speedup vs baseline: 1.7026x; 1.0319x over previous
"""Bass/Trainium2 kernel for the BilinearInteractionLayer problem.

out[b, p, f] = (sum_e emb[b, I[p], e] * W[p, f, e]) * emb[b, J[p], f]
  emb: [2048, 40, 64] f32, W: [780, 64, 64] f32, out: [2048, 780, 64] f32

Strategy (data parallel over batch, 8 cores x 256 rows), bf16 compute:
  - All operands cast to bf16 on host; matmul accumulates fp32 in PSUM; the
    output is written bf16 and upcast to fp32 on host.  This halves every
    HBM stream (out 25.6 MB, W 6.4 MB, emb 2.6 MB per core) and the kernel
    is HBM-bound, so bytes ~= time.
  - Pairs (i, j) grouped by i ("blocks"; block i has 39-i pairs, consecutive p).
    Blocks split into two 390-pair halves (A: i in 0..9 + 30..38, B: i in
    10..29) assigned to PE row-groups 0-63 / 64-127 so two K=64 matmuls can
    overlap on the 128x128 array.
  - Per half, a "tape" of 390*64 = 24960 (pair, f) columns; W is pre-arranged
    on host to [128, 24960] bf16 (partition = e; half A rows 0-63, half B rows
    64-127) and streamed in 6 chunks of 4160 cols (~1 MB DMAs).
  - matmul: lhsT = embT[e, b] (stationary, [64, 128] per batch-chunk),
    rhs = W chunk slice [64, <=512], out psum[b, (pair, f)] fp32; halves
    alternate so one half's LDWEIGHTS overlaps the other's pipe drain.
  - Epilogue split across three engines so no engine exceeds the DMA floor,
    assigned per block-run by a deficit balance: a run either gets DVE
    tensor_muls straight out of PSUM (fp32 x bf16 embN -> bf16 out tile), or
    ACT casts (PSUM -> bf16 out tile) with one merged in-place GpSimd bf16
    multiply; the last run of each out tile goes to DVE so the store isn't
    gated by a late GpSimd flush.
  - Out tiles [128, 4160] bf16 DMA to HBM in tape order; host reorders
    tape pair order -> global pair order and upcasts.
"""

import os
import numpy as np
import ml_dtypes

import concourse.mybir as mybir
import concourse.tile as tile
from concourse import bacc
from concourse import bass_utils

F32 = mybir.dt.float32
BF16 = mybir.dt.bfloat16
NPBF16 = ml_dtypes.bfloat16

NUM_FIELDS = 40
EMBED = 64
BATCH = 2048
NCORES = 8
BL = BATCH // NCORES          # 256 rows per core
BCHUNKS = 2                   # 2 x 128 partition chunks of the local batch
NPAIRS = 780

HALVES = [list(range(0, 10)) + list(range(30, 39)), list(range(10, 30))]
HALF_PAIRS = 390
TAPE = HALF_PAIRS * EMBED     # 24960 cols per half
CHUNK = 4160                  # W/out tile width (cols); 6 even chunks
NCHUNK = TAPE // CHUNK
PSGRID = 1024                 # psum tile width (2 banks, fp32)
MMMAX = 512                   # max matmul free dim (one psum bank, fp32 out)

# Epilogue path split (fractions of output columns), tuned from measured
# engine rates: DVE direct-psum multiply ~1.3 ns/col, ACT cast ~1.1,
# GpSimd bf16 multiply ~2.05 (the GPS path also costs an ACT cast).
TARGET = {"dve": 0.625, "gps": 0.375}


def _half_blocks(h):
    """[(i, tape_start_col, ncols)] for half h, in tape order."""
    res = []
    pos = 0
    for i in HALVES[h]:
        cols = (NUM_FIELDS - 1 - i) * EMBED
        res.append((i, pos, cols))
        pos += cols
    assert pos == TAPE
    return res


def _chunk_groups(h, c):
    """Groups for chunk c of half h: (i, abs_start, cols, j0).

    Split at block boundaries and at the PSGRID grid (relative to the chunk
    start) so each group fits one psum tile; j0 is the first j of the group.
    All boundaries are multiples of 64.
    """
    c0, c1 = c * CHUNK, (c + 1) * CHUNK
    groups = []
    for (i, b0, bcols) in _half_blocks(h):
        lo, hi = max(b0, c0), min(b0 + bcols, c1)
        s = lo
        while s < hi:
            nxt = c0 + ((s - c0) // PSGRID + 1) * PSGRID
            e = min(hi, nxt)
            j0 = i + 1 + (s - b0) // EMBED
            groups.append((i, s, e - s, j0))
            s = e
    return groups


def _chunk_runs(h, c):
    """Pieces of chunk c of half h grouped into per-block runs.

    Returns [ [ (i, abs_start, cols, j0), ... ], ... ] where each inner list
    is the PSGRID-split pieces of one block(cap chunk) in tape order (so the
    run's columns are contiguous in both the out tile and embN).
    """
    runs = []
    for g in _chunk_groups(h, c):
        if runs and runs[-1][0][0] == g[0]:
            runs[-1].append(g)
        else:
            runs.append([g])
    return runs


def _pairs_tape():
    """Global pair indices (combinations order) in tape order: half A then B."""
    pidx = {}
    k = 0
    for i in range(NUM_FIELDS):
        for j in range(i + 1, NUM_FIELDS):
            pidx[(i, j)] = k
            k += 1
    order = []
    for h in (0, 1):
        for i in HALVES[h]:
            for j in range(i + 1, NUM_FIELDS):
                order.append(pidx[(i, j)])
    return np.array(order, dtype=np.int64)


def _build_nc():
    nc = bacc.Bacc("TRN2", target_bir_lowering=False, debug=False)

    wt_d = nc.dram_tensor("Wt", [128, TAPE], BF16, kind="ExternalInput")
    # embT is staged host-side duplicated into both partition halves so a
    # single full-128-partition DMA loads it (the compiler requires lhsT and
    # rhs on the same partitions, and 64-partition DMAs run at half rate).
    embT_d = nc.dram_tensor("embT", [128, NUM_FIELDS * BL], BF16,
                            kind="ExternalInput")
    embN_d = nc.dram_tensor("embN", [128, BCHUNKS * NUM_FIELDS * EMBED], BF16,
                            kind="ExternalInput")
    out_d = nc.dram_tensor("out", [BL, 2 * TAPE], BF16, kind="ExternalOutput")

    wt_ap, embT_ap, embN_ap, out_ap = (
        wt_d.ap(), embT_d.ap(), embN_d.ap(), out_d.ap())

    NF = NUM_FIELDS * EMBED  # 2560, embN cols per batch chunk

    with tile.TileContext(nc) as tc:
        with (
            tc.tile_pool(name="const", bufs=1) as cpool,
            tc.tile_pool(name="w", bufs=4) as wpool,
            tc.tile_pool(name="o", bufs=8) as opool,
            tc.tile_pool(name="ps", bufs=4, space="PSUM") as ppool,
        ):
            # Startup: load W chunk 0 and embT in small "starter" segments so
            # the first matmuls fire as soon as ~1 MB has landed, instead of
            # waiting for the full 2.3 MB.  Ring order interleaves W and embT
            # segments in first-use order; embN rides the scalar ring in
            # parallel.  W chunk c+2 is issued ahead of chunk c's out stores
            # for a one-chunk prefetch runway (the interleaved loads also
            # fill store-dependency gaps on the ring).
            hc = NUM_FIELDS * BL // 2
            w0a = wpool.tile([128, 2048], BF16, tag="w", name="w0a")
            nc.sync.dma_start(w0a[:], wt_ap[:, :2048])
            eTa = cpool.tile([128, 1024], BF16, name="eTa")
            nc.sync.dma_start(eTa[:], embT_ap[:, :1024])
            embN_s = cpool.tile([128, BCHUNKS * NF], BF16)
            nc.scalar.dma_start(embN_s[:], embN_ap[:])
            w0b = wpool.tile([128, CHUNK - 2048], BF16, tag="w", name="w0b")
            nc.sync.dma_start(w0b[:], wt_ap[:, 2048:CHUNK])
            eTb = cpool.tile([128, hc - 1024], BF16, name="eTb")
            nc.sync.dma_start(eTb[:], embT_ap[:, 1024:hc])
            wts = {0: [(w0a, 0, 2048), (w0b, 2048, CHUNK)]}
            wts[1] = [(wpool.tile([128, CHUNK], BF16, tag="w", name="wt"),
                       0, CHUNK)]
            nc.sync.dma_start(wts[1][0][0][:], wt_ap[:, CHUNK:2 * CHUNK])
            eTc = cpool.tile([128, hc], BF16, name="eTc")
            nc.sync.dma_start(eTc[:], embT_ap[:, hc:])
            embT_segs = [(eTa, 0, 1024), (eTb, 1024, hc), (eTc, hc, 2 * hc)]

            def lhsT_slice(h, col0):
                for (t, lo, hi) in embT_segs:
                    if lo <= col0 < hi:
                        return t[h * 64:(h + 1) * 64, col0 - lo:col0 - lo + 128]
                raise AssertionError(col0)

            def wt_slice(segs, h, lo, width):
                for (t, slo, shi) in segs:
                    if slo <= lo < shi:
                        return t[h * 64:(h + 1) * 64, lo - slo:lo - slo + width]
                raise AssertionError(lo)

            # Deficit state for the epilogue path balance.
            got = {k: 0.0 for k in TARGET}
            tot = [0.0]

            for c in range(NCHUNK):
                wt_segs = wts.pop(c)
                if c + 2 < NCHUNK:
                    wnew = wpool.tile([128, CHUNK], BF16, tag="w", name="wt")
                    wts[c + 2] = [(wnew, 0, CHUNK)]
                    nc.sync.dma_start(wnew[:],
                                      wt_ap[:, (c + 2) * CHUNK:(c + 3) * CHUNK])
                if c == NCHUNK - 1:
                    # final chunk: break run-merging so pieces spread across
                    # engines at fine grain and the last tiles finish sooner
                    runs_h = [[[g] for g in _chunk_groups(0, c)],
                              [[g] for g in _chunk_groups(1, c)]]
                else:
                    runs_h = [_chunk_runs(0, c), _chunk_runs(1, c)]
                for bc in range(BCHUNKS):
                    otiles = [opool.tile([128, CHUNK], BF16, tag="o",
                                         name=f"o{c}_{bc}_{h}")
                              for h in range(2)]

                    def emit_half(h):
                        # Global deficit balance toward TARGET fractions so
                        # DVE and the ACT+GpSimd chain run concurrently; the
                        # last run of each out tile is forced to DVE so the
                        # tile's store isn't gated by a late GpSimd flush.
                        nruns = len(runs_h[h])
                        for ridx, run in enumerate(runs_h[h]):
                            rcols = sum(g[2] for g in run)
                            tot[0] += rcols
                            if ridx == nruns - 1:
                                path = "dve"
                            else:
                                path = max(TARGET, key=lambda k:
                                           TARGET[k] * tot[0] - got[k])
                            got[path] += rcols
                            for (i, gs, gcols, j0) in run:
                                pt = ppool.tile([128, PSGRID], F32, tag="ps",
                                                name=f"ps{c}_{bc}_{h}_{gs}")
                                s = 0
                                while s < gcols:
                                    w = min(MMMAX, gcols - s)
                                    yield ("mm", (h, pt, i, gs, s, w))
                                    s += w
                                yield ("epi", (h, pt, gs, gcols, j0, path))
                            if path != "dve":
                                yield ("flush",
                                       (h, run[0][1], rcols, run[0][3]))

                    streams = [emit_half(0), emit_half(1)]
                    done = [False, False]
                    turn = 0
                    while not all(done):
                        if done[turn]:
                            turn ^= 1
                        try:
                            kind, args = next(streams[turn])
                        except StopIteration:
                            done[turn] = True
                            turn ^= 1
                            continue
                        if kind == "mm":
                            h, pt, i, gs, s, w = args
                            rel = gs - c * CHUNK
                            col0 = i * BL + bc * 128
                            nc.tensor.matmul(
                                pt[:, s:s + w],
                                lhsT=lhsT_slice(h, col0),
                                rhs=wt_slice(wt_segs, h, rel + s, w),
                                start=True, stop=True,
                            )
                            # alternate halves between matmuls so LDWEIGHTS
                            # for one PE row-group overlaps the other's drain
                            turn ^= 1
                        elif kind == "epi":
                            h, pt, gs, gcols, j0, path = args
                            rel = gs - c * CHUNK
                            ncol = bc * NF + j0 * EMBED
                            ot = otiles[h]
                            if path == "dve":
                                nc.vector.tensor_mul(
                                    ot[:, rel:rel + gcols],
                                    pt[:, :gcols],
                                    embN_s[:, ncol:ncol + gcols],
                                )
                            else:
                                nc.scalar.copy(
                                    ot[:, rel:rel + gcols],
                                    pt[:, :gcols],
                                )
                        else:
                            h, gs0, rcols, j0 = args
                            rel = gs0 - c * CHUNK
                            ncol = bc * NF + j0 * EMBED
                            ot = otiles[h]
                            nc.gpsimd.tensor_mul(
                                ot[:, rel:rel + rcols],
                                ot[:, rel:rel + rcols],
                                embN_s[:, ncol:ncol + rcols],
                            )
                    for h in range(2):
                        # out stores live on the sync ring (an issue on the
                        # ACT ring would head-of-line block later casts);
                        # the final chunk's tiles split at a piece boundary
                        # across both rings, so each half drains as soon as
                        # its writers finish and the tail shrinks.
                        dst = out_ap[bc * 128:(bc + 1) * 128,
                                     h * TAPE + c * CHUNK:
                                     h * TAPE + (c + 1) * CHUNK]
                        if c == NCHUNK - 1:
                            hw = 2048
                            nc.sync.dma_start(dst[:, :hw], otiles[h][:, :hw])
                            nc.scalar.dma_start(dst[:, hw:], otiles[h][:, hw:])
                        else:
                            nc.sync.dma_start(dst, otiles[h][:])

    nc.compile()
    return nc


_NC = None
_TAPE_ORDER = None
LAST_RESULT = None


def kernel(feature_emb, W):
    global _NC, _TAPE_ORDER, LAST_RESULT
    feature_emb = np.ascontiguousarray(feature_emb, dtype=np.float32)
    W = np.ascontiguousarray(W, dtype=np.float32)
    assert feature_emb.shape == (BATCH, NUM_FIELDS, EMBED)
    assert W.shape == (NPAIRS, EMBED, EMBED)

    if _NC is None:
        _NC = _build_nc()
        _TAPE_ORDER = _pairs_tape()

    # W tape: [128, 24960] bf16; rows 0-63 half A (partition = e), 64-127 half B
    wsel = W[_TAPE_ORDER]                       # [780, 64(f), 64(e)] tape order
    wa = wsel[:HALF_PAIRS].transpose(2, 0, 1).reshape(EMBED, TAPE)
    wb = wsel[HALF_PAIRS:].transpose(2, 0, 1).reshape(EMBED, TAPE)
    wt = np.ascontiguousarray(
        np.concatenate([wa, wb], axis=0)).astype(NPBF16)

    in_maps = []
    for c in range(NCORES):
        ec = feature_emb[c * BL:(c + 1) * BL]   # [256, 40, 64]
        embT1 = ec.transpose(2, 1, 0).reshape(EMBED, NUM_FIELDS * BL)
        embT = np.ascontiguousarray(
            np.concatenate([embT1, embT1], axis=0)).astype(NPBF16)
        embN = np.ascontiguousarray(
            ec.reshape(BCHUNKS, 128, NUM_FIELDS * EMBED)
              .transpose(1, 0, 2)
              .reshape(128, BCHUNKS * NUM_FIELDS * EMBED)).astype(NPBF16)
        in_maps.append({"Wt": wt, "embT": embT, "embN": embN})

    trace = bool(int(os.environ.get("BILIN_TRACE", "0")))
    res = bass_utils.run_bass_kernel_spmd(
        _NC, in_maps, core_ids=list(range(NCORES)), trace=trace)
    LAST_RESULT = res

    out = np.empty((BATCH, NPAIRS, EMBED), dtype=np.float32)
    for c in range(NCORES):
        t = np.asarray(res.results[c]["out"]).astype(np.float32)
        t = t.reshape(BL, NPAIRS, EMBED)
        out[c * BL:(c + 1) * BL][:, _TAPE_ORDER, :] = t
    return out
